# revision 1
# baseline (speedup 1.0000x reference)
"""CNN+LSTM seq2seq kernel for trn2, 8-core data parallel.

Model (per core, batch 64):
  conv1d(16->64, k=5, same) -> relu -> maxpool2 -> LSTM(64->512) over 512 steps
  -> autoregressive LSTM(1->512) decoder 64 steps with linear head(512->1).

Sharding: batch 512 split across 8 cores; weights replicated. No collectives.
"""

import numpy as np

import concourse.bass as bass
import concourse.mybir as mybir
import concourse.tile as tile_mod
from concourse import bacc
from concourse.bass import ds, ts
from concourse.masks import make_identity

F32 = mybir.dt.float32
AF = mybir.ActivationFunctionType

B = 64        # batch per core
S = 1024      # input seq len
CIN = 16
OC = 64       # conv out channels
KW = 5
T2 = 512      # encoder steps after pool
H = 512       # hidden
G = 4 * H     # gates
OUT_STEPS = 64
NCORES = 8

# gate layout [g f i o] (torch blocks permuted): tanh(g) can issue as soon as
# bank 0 finishes accumulating, sigma(f,i) next, and only sigma(o) -> h ->
# transpose trails the last matmul chunk
PERM = [8, 9, 10, 11, 4, 5, 6, 7, 0, 1, 2, 3, 12, 13, 14, 15]

F32R = mybir.dt.float32r


def _mmr(nc, out, lhsT, rhs, start, stop):
    """fp32 matmul streamed in float32r mode (1 cycle/row vs 4 for fp32)."""
    nc.tensor.matmul(out, lhsT=lhsT.bitcast(F32R), rhs=rhs.bitcast(F32R),
                     start=start, stop=stop)


PHASES = {}


def _mark(nc, name):
    PHASES[name] = int(nc.get_next_instruction_name().split("-")[-1])


def build_nc(repeat=1, variant="full"):
    skip_conv = "noconv" in variant
    skip_elem = "mm" in variant
    skip_tr = "noT" in variant or skip_elem
    skip_xdma = "nodma" in variant
    nc = bacc.Bacc(None, target_bir_lowering=False, debug=False)

    # ---------- DRAM I/O ----------
    x_d = nc.dram_tensor("x", [B, S, CIN], F32, kind="ExternalInput")
    dstart_d = nc.dram_tensor("decoder_start", [B, 1], F32, kind="ExternalInput")
    convw_d = nc.dram_tensor("conv_w", [OC, CIN, KW], F32, kind="ExternalInput")
    convb_d = nc.dram_tensor("conv_b", [OC], F32, kind="ExternalInput")
    encWih_d = nc.dram_tensor("enc_Wih", [G, OC], F32, kind="ExternalInput")
    encWhh_d = nc.dram_tensor("enc_Whh", [G, H], F32, kind="ExternalInput")
    encb_d = nc.dram_tensor("enc_b", [G], F32, kind="ExternalInput")
    decWih_d = nc.dram_tensor("dec_Wih", [G, 1], F32, kind="ExternalInput")
    decWhh_d = nc.dram_tensor("dec_Whh", [G, H], F32, kind="ExternalInput")
    decb_d = nc.dram_tensor("dec_b", [G], F32, kind="ExternalInput")
    headw_d = nc.dram_tensor("head_w", [1, H], F32, kind="ExternalInput")
    headb_d = nc.dram_tensor("head_b", [1], F32, kind="ExternalInput")
    out_d = nc.dram_tensor("out", [B, OUT_STEPS], F32, kind="ExternalOutput")

    with tile_mod.TileContext(nc) as tc:
        with tc.tile_pool(name="dram", bufs=1, space="DRAM") as dramp:
            # encoder inputs staged in DRAM as [t', b, oc] so the per-step
            # stationary load is one contiguous 16KB block
            enc_x = dramp.tile([T2, B, OC], F32)

            with tc.tile_pool(name="const", bufs=1) as cn:
                identity = cn.tile([128, 128], F32)
                make_identity(nc, identity)
                id64 = identity[:64, :64]

                # persistent weights (stream operands)
                hW = [cn.tile([128, G], F32, name=f"hW{k}") for k in range(4)]
                xW = cn.tile([OC + 1, G], F32)          # rows 0..63 Wih.T, row 64 enc_b
                dhW = [cn.tile([128, G], F32, name=f"dhW{k}") for k in range(4)]
                dxW = cn.tile([2, G], F32)              # row0 dec_Wih.T, row1 dec_b
                cwT = [cn.tile([CIN, OC], F32, name=f"cwT{k}") for k in range(KW)]
                cb = cn.tile([OC, 1], F32)
                hdT = cn.tile([128, 4], F32)            # head_w.T chunks as columns
                hb = cn.tile([1, 1], F32)
                ones_row = cn.tile([1, B], F32)
                nc.vector.memset(ones_row, 1.0)
                zpad = cn.tile([CIN, 2], F32)
                nc.vector.memset(zpad, 0.0)

                # persistent state
                c_st = cn.tile([B, H], F32)
                hT = cn.tile([128, 4 * B], F32)         # h.T, chunk k at [:, 64k:64k+64]
                sig = cn.tile([B, 3 * H], F32)          # sigmoid(f,i,o)
                tg = cn.tile([B, H], F32)
                tcell = cn.tile([B, H], F32)
                h_st = cn.tile([B, H], F32)
                t1 = cn.tile([B, H], F32)
                t2 = cn.tile([B, H], F32)
                outF = cn.tile([B, OUT_STEPS], F32)

                # ---------- weight prep (on-chip transposes) ----------
                with (
                    tc.tile_pool(name="wtmp", bufs=3) as wt,
                    tc.tile_pool(name="wps", bufs=3, space="PSUM") as wps,
                ):
                    def prep_whh(src_d, dst_tiles):
                        for jb in range(16):
                            n = PERM.index(jb)
                            wtmp = wt.tile([128, H], F32, tag="wtmp")
                            nc.sync.dma_start(out=wtmp, in_=src_d[128 * jb:128 * (jb + 1), :])
                            for kc in range(4):
                                wtp = wps.tile([128, 128], F32, tag="wtp")
                                nc.tensor.transpose(wtp, wtmp[:, 128 * kc:128 * (kc + 1)], identity)
                                eng = nc.scalar if (kc % 2 == 0) else nc.vector
                                dst = dst_tiles[kc][:, 128 * n:128 * (n + 1)].bitcast(F32R)
                                if eng is nc.scalar:
                                    nc.scalar.copy(dst, wtp)
                                else:
                                    nc.vector.tensor_copy(dst, wtp)

                    prep_whh(encWhh_d, hW)
                    prep_whh(decWhh_d, dhW)

                    # enc_Wih.T into xW rows 0..63
                    for jb in range(16):
                        n = PERM.index(jb)
                        wtmp2 = wt.tile([128, OC], F32, tag="wtmp2")
                        nc.sync.dma_start(out=wtmp2, in_=encWih_d[128 * jb:128 * (jb + 1), :])
                        wtp = wps.tile([128, 128], F32, tag="wtp")
                        nc.tensor.transpose(wtp[:OC, :128], wtmp2, identity)
                        nc.scalar.copy(xW[0:OC, 128 * n:128 * (n + 1)].bitcast(F32R), wtp[:OC, :128])
                    # biases / vectors: DMA into fp32 staging, then engine
                    # copies round them to f32r for the matmul stream
                    bstage = wt.tile([1, G], F32, tag="bstage", bufs=1)
                    dstage = wt.tile([2, G], F32, tag="dstage", bufs=1)
                    for n in range(16):
                        jb = PERM[n]
                        nc.sync.dma_start(out=bstage[:, 128 * n:128 * (n + 1)],
                                          in_=encb_d[None, 128 * jb:128 * (jb + 1)])
                        nc.sync.dma_start(out=dstage[0:1, 128 * n:128 * (n + 1)],
                                          in_=decWih_d[128 * jb:128 * (jb + 1), :].rearrange("a b -> b a"))
                        nc.sync.dma_start(out=dstage[1:2, 128 * n:128 * (n + 1)],
                                          in_=decb_d[None, 128 * jb:128 * (jb + 1)])
                    nc.scalar.copy(xW[OC:OC + 1, :].bitcast(F32R), bstage)
                    nc.vector.tensor_copy(dxW[:, :].bitcast(F32R), dstage)
                    # conv weights: cwT[k][ic, oc] = conv_w[oc, ic, k]
                    cstage = wt.tile([CIN, KW * OC], F32, tag="cstage", bufs=1)
                    for k in range(KW):
                        nc.sync.dma_start(
                            out=cstage[:, OC * k:OC * (k + 1)],
                            in_=convw_d[:, :, k].rearrange("oc ic -> ic oc"),
                        )
                        nc.scalar.copy(cwT[k][:, :].bitcast(F32R), cstage[:, OC * k:OC * (k + 1)])
                    nc.sync.dma_start(out=cb, in_=convb_d[:, None])
                    # head_w.T chunks as columns of hdT
                    for kc in range(4):
                        nc.sync.dma_start(
                            out=hdT[:, kc:kc + 1],
                            in_=headw_d[:, 128 * kc:128 * (kc + 1)].rearrange("a b -> b a"),
                        )
                    nc.sync.dma_start(out=hb, in_=headb_d[:, None])

                _mark(nc, "conv_start")
                for _rep in range(repeat):
                  nc.vector.memset(c_st, 0.0)
                  nc.vector.memset(hT, 0.0)
                  if skip_elem:
                      for _tl in (sig, tg, tcell, h_st, t1, t2):
                          nc.vector.memset(_tl, 0.0)
                  # ---------- conv + pool -> enc_x ----------
                  with (
                      tc.tile_pool(name="conv", bufs=2) as cp,
                      tc.tile_pool(name="convps", bufs=2, space="PSUM") as cpp,
                  ):
                      for b in ([] if skip_conv else range(B)):
                          # xTb rows 0:16 hold x[b].T with 2-col zero pads; rows
                          # 16:32 are scratch written by the 32-partition unpack
                          # copies (PSUM reads must start 32-aligned).
                          xTb = cp.tile([32, S + 4 + 4], F32, tag="xTb")
                          nc.vector.tensor_copy(xTb[0:CIN, 0:2].bitcast(F32R), zpad)
                          nc.vector.tensor_copy(xTb[0:CIN, 2 + S:2 + S + 2].bitcast(F32R), zpad)
                          for half in range(2):
                              xb_raw = cp.tile([128, 128], F32, tag="xb_raw", bufs=3)
                              nc.sync.dma_start(
                                  out=xb_raw.rearrange("p (a c) -> p a c", c=32)[:, :, 0:CIN],
                                  in_=x_d[b].rearrange("(a p) c -> p a c", p=128)[:, 4 * half:4 * half + 4, :],
                              )
                              xtp = cpp.tile([128, 128], F32, tag="xtp")
                              nc.tensor.transpose(xtp, xb_raw, identity)
                              for a in range(4):
                                  blk = xtp[32 * a:32 * (a + 1), :]
                                  dst = xTb[:, 2 + 128 * (4 * half + a):2 + 128 * (4 * half + a + 1)].bitcast(F32R)
                                  if a % 2 == 0:
                                      nc.scalar.copy(dst, blk)
                                  else:
                                      nc.vector.tensor_copy(dst, blk)
                          yb = cp.tile([OC, S], F32, tag="yb")
                          for half in range(2):
                              cps = cpp.tile([OC, 512], F32, tag="cps")
                              for k in range(KW):
                                  _mmr(nc, cps, cwT[k],
                                       xTb[0:CIN, k + 512 * half:k + 512 * half + 512],
                                       k == 0, k == KW - 1)
                              nc.scalar.activation(yb[:, 512 * half:512 * (half + 1)], cps,
                                                   AF.Relu, bias=cb[:, 0:1])
                          pooled = cp.tile([OC, T2], F32, tag="pooled")
                          yb_pairs = yb.rearrange("p (t two) -> p t two", two=2)
                          nc.vector.tensor_max(pooled, yb_pairs[:, :, 0], yb_pairs[:, :, 1])
                          poolT = cp.tile([128, 4 * OC], F32, tag="poolT")
                          for q in range(4):
                              ptp = cpp.tile([128, OC], F32, tag="ptp")
                              nc.tensor.transpose(ptp, pooled[:, 128 * q:128 * (q + 1)], id64)
                              if q % 2 == 0:
                                  nc.scalar.copy(poolT[:, OC * q:OC * (q + 1)], ptp)
                              else:
                                  nc.vector.tensor_copy(poolT[:, OC * q:OC * (q + 1)], ptp)
                          for q in range(4):
                              nc.sync.dma_start(
                                  out=enc_x[128 * q:128 * (q + 1), b, :],
                                  in_=poolT[:, OC * q:OC * (q + 1)],
                              )

                  _mark(nc, "enc_start")
                  # ---------- encoder + decoder ----------
                  with (
                      tc.tile_pool(name="step", bufs=2) as sp,
                      tc.tile_pool(name="lps", bufs=1, space="PSUM") as lp,
                  ):
                      gps = lp.tile([B, G], F32, tag="gates")

                      HH = H // 2

                      def emit_tr(qs):
                          """Transpose h_st 128-col chunks qs into hT (f32r);
                          one combined DVE copy per pair."""
                          htp = lp.tile([128, 2 * B], F32, tag="htp", bufs=2)
                          for j, q in enumerate(qs):
                              nc.tensor.transpose(htp[:, B * j:B * (j + 1)],
                                                  h_st[:, 128 * q:128 * (q + 1)], id64)
                          q0 = qs[0]
                          nc.vector.tensor_copy(
                              hT[:, B * q0:B * (q0 + 2)].bitcast(F32R), htp)

                      def lstm_elementwise(gps_, defer_q23):
                          if skip_elem:
                              return
                          """gates psum [g f i o] -> updates c_st, h_st, hT.

                          tanh(g), sigma(f,i,o), t1/t2/c' overlap the i/o matmul
                          chunks. The h chain is split in hidden halves; with
                          defer_q23 the second half's transposes are NOT emitted
                          here (the caller emits them next step, after the k0/k1
                          matmuls, to avoid PE head-of-line blocking)."""
                          nc.scalar.activation(tg, gps_[:, 0:H], AF.Tanh)
                          nc.scalar.activation(sig[:, 0:2 * H], gps_[:, H:3 * H], AF.Sigmoid)
                          nc.scalar.activation(sig[:, 2 * H:3 * H], gps_[:, 3 * H:4 * H], AF.Sigmoid)
                          # keep-warm: HAM re-throttles PE after ~3.4us idle;
                          # a dummy matmul ordered after the sigma(o) read fires
                          # mid-tail and keeps the array active (dead region of
                          # bank 3, overwritten by next step's start=True MM)
                          nc.tensor.matmul(gps_[0:1, 3 * H + B:3 * H + 2 * B],
                                           lhsT=ones_row[:, 0:1], rhs=ones_row,
                                           start=True, stop=True)
                          nc.vector.tensor_mul(t1, sig[:, 0:H], c_st)
                          nc.vector.tensor_mul(t2, sig[:, H:2 * H], tg)
                          nc.vector.tensor_add(c_st, t1, t2)
                          for hh in range(2):
                              sl = slice(HH * hh, HH * (hh + 1))
                              nc.scalar.activation(tcell[:, sl], c_st[:, sl], AF.Tanh)
                              nc.vector.tensor_mul(h_st[:, sl], sig[:, 2 * H + HH * hh:2 * H + HH * (hh + 1)], tcell[:, sl])
                              if skip_tr:
                                  continue
                              if hh == 0:
                                  emit_tr((0, 1))
                                  # second keep-warm beat, gated on the fresh
                                  # hT copy so it lands late in the tail
                                  nc.tensor.matmul(gps_[0:1, 3 * H + 2 * B:3 * H + 3 * B],
                                                   lhsT=hT[:, 0:1].bitcast(F32R),
                                                   rhs=hW[0][:, 0:B].bitcast(F32R),
                                                   start=True, stop=True)
                              elif not defer_q23:
                                  emit_tr((2, 3))

                      # --- encoder loop (static unroll, software-pipelined:
                      # step t-1's q2/q3 transposes are emitted between step t's
                      # k0/k1 and k2/k3 matmul groups) ---
                      xsT_fix = None
                      if skip_xdma:
                          xsT_fix = sp.tile([OC + 1, B], F32, tag="xsTf", bufs=1)
                          nc.vector.memset(xsT_fix.bitcast(F32R), 0.01)
                      for t in range(T2):
                          if skip_xdma:
                              xsT = xsT_fix
                          else:
                              xb = sp.tile([B, OC + 1], F32, tag="xb", bufs=4)
                              nc.sync.dma_start(out=xb[:, 0:OC], in_=enc_x[t])
                              nc.gpsimd.memset(xb[:, OC:OC + 1], 1.0)
                              xps = lp.tile([OC + 1, B], F32, tag="small", bufs=2)
                              nc.tensor.transpose(xps, xb, id64)
                              xsT = sp.tile([OC + 1, B], F32, tag="xsT", bufs=3)
                              nc.vector.tensor_copy(xsT.bitcast(F32R), xps)
                          for n in range(4):
                              gsl = gps[:, 512 * n:512 * (n + 1)]
                              _mmr(nc, gsl, xsT, xW[:, 512 * n:512 * (n + 1)], True, False)
                              for kc in range(2):
                                  _mmr(nc, gsl, hT[:, B * kc:B * (kc + 1)],
                                       hW[kc][:, 512 * n:512 * (n + 1)], False, False)
                          if t > 0 and not skip_tr:
                              emit_tr((2, 3))
                          for n in range(4):
                              gsl = gps[:, 512 * n:512 * (n + 1)]
                              for kc in range(2, 4):
                                  _mmr(nc, gsl, hT[:, B * kc:B * (kc + 1)],
                                       hW[kc][:, 512 * n:512 * (n + 1)], False, kc == 3)
                          lstm_elementwise(gps, defer_q23=True)
                      emit_tr((2, 3))

                      _mark(nc, "dec_start")
                      # --- decoder prep ---
                      dssb = sp.tile([B, 2], F32, tag="dssb", bufs=1)
                      nc.sync.dma_start(out=dssb[:, 0:1], in_=dstart_d[:, :])
                      nc.gpsimd.memset(dssb[:, 1:2], 1.0)
                      dsps = lp.tile([2, B], F32, tag="small", bufs=2)
                      nc.tensor.transpose(dsps, dssb, id64)
                      aug = sp.tile([2, B], F32, tag="aug", bufs=3)
                      nc.vector.tensor_copy(aug.bitcast(F32R), dsps)

                      # --- decoder loop ---
                      for t in range(OUT_STEPS):
                          for n in range(4):
                              gsl = gps[:, 512 * n:512 * (n + 1)]
                              _mmr(nc, gsl, aug, dxW[:, 512 * n:512 * (n + 1)], True, False)
                              for kc in range(4):
                                  _mmr(nc, gsl, hT[:, B * kc:B * (kc + 1)],
                                       dhW[kc][:, 512 * n:512 * (n + 1)], False, kc == 3)
                          lstm_elementwise(gps, defer_q23=False)
                          # head: pred.T = head_w @ h.T + head_b
                          hps = lp.tile([1, B], F32, tag="small", bufs=2)
                          for kc in range(4):
                              nc.tensor.matmul(hps, lhsT=hdT[:, kc:kc + 1],
                                               rhs=hT[:, B * kc:B * (kc + 1)],
                                               start=(kc == 0), stop=False)
                          nc.tensor.matmul(hps, lhsT=hb, rhs=ones_row,
                                           start=False, stop=True)
                          predsb = sp.tile([1, B], F32, tag="predsb", bufs=2)
                          nc.scalar.copy(predsb, hps)
                          opc = lp.tile([B, 1], F32, tag="small", bufs=2)
                          nc.tensor.transpose(opc, predsb, identity[:1, :1])
                          nc.scalar.copy(outF[:, t:t + 1], opc)
                          if t + 1 < OUT_STEPS:
                              augsb = sp.tile([B, 2], F32, tag="augsb", bufs=2)
                              nc.vector.tensor_copy(augsb[:, 0:1], opc)
                              nc.gpsimd.memset(augsb[:, 1:2], 1.0)
                              augps = lp.tile([2, B], F32, tag="small", bufs=2)
                              nc.tensor.transpose(augps, augsb, id64)
                              aug = sp.tile([2, B], F32, tag="aug", bufs=3)
                              nc.vector.tensor_copy(aug.bitcast(F32R), augps)

                      nc.sync.dma_start(out=out_d[:, :], in_=outF)

    _mark(nc, "end")
    nc.compile()
    return nc


_CACHED = {}


def kernel(**inputs):
    """Full-input entry: shard batch across 8 cores, run SPMD, gather."""
    from concourse.bass_utils import run_bass_kernel_spmd

    if "nc" not in _CACHED:
        _CACHED["nc"] = build_nc()
    nc = _CACHED["nc"]

    full = {k: np.ascontiguousarray(np.asarray(v, dtype=np.float32)) for k, v in inputs.items()}
    per_core = []
    for c in range(NCORES):
        sl = slice(c * B, (c + 1) * B)
        m = {}
        for k, v in full.items():
            if k in ("x", "decoder_start"):
                m[k] = np.ascontiguousarray(v[sl])
            else:
                m[k] = v
        per_core.append(m)

    res = run_bass_kernel_spmd(nc, per_core, core_ids=list(range(NCORES)))
    outs = [r["out"] for r in res.results]
    return np.concatenate(outs, axis=0)



# revision 25
# speedup vs baseline: 1.1544x; 1.1544x over previous
"""CNN+LSTM seq2seq kernel for trn2, 8-core data parallel.

Model (per core, batch 64):
  conv1d(16->64, k=5, same) -> relu -> maxpool2 -> LSTM(64->512) over 512 steps
  -> autoregressive LSTM(1->512) decoder 64 steps with linear head(512->1).

Sharding: batch 512 split across 8 cores; weights replicated. No collectives.

Structure notes:
 - gate matmuls are grouped by stationary operand (kc-major) so consecutive
   matmuls share lhsT.
 - hT is double buffered; both transpose pairs (q01/q23) of step t are
   emitted inside step t+1's matmul stream to avoid PE head-of-line blocks.
 - decoder: pred feedback is folded into the recurrent weights
   (dWhh' = dWhh.T + head_w (x) dWih, db' = db + head_b*dWih), so the serial
   pred->aug->matmul chain disappears; pred itself is computed on DVE with a
   fused multiply-reduce straight into the output tile. Step 0 uses
   aug0 = [dstart - pred_enc; 1] against [dWih; db'] to correct the fold.
"""

import numpy as np

import concourse.bass as bass
import concourse.mybir as mybir
import concourse.tile as tile_mod
from concourse import bacc
from concourse.masks import make_identity

F32 = mybir.dt.float32
F32R = mybir.dt.float32r
AF = mybir.ActivationFunctionType
ALU = mybir.AluOpType

B = 64        # batch per core
S = 1024      # input seq len
CIN = 16
OC = 64       # conv out channels
KW = 5
T2 = 512      # encoder steps after pool
H = 512       # hidden
HH = 256
G = 4 * H     # gates
OUT_STEPS = 64
NCORES = 8
NXB = 4       # x-slot rotation depth

# gate bank layout [g f i o] (torch blocks [i f g o] permuted)
PERM = [8, 9, 10, 11, 4, 5, 6, 7, 0, 1, 2, 3, 12, 13, 14, 15]

# col-packing: two gate chunks per PSUM bank via tile_position col-tiling.
# bank A = [g; o], bank B = [f; i]; the two 64-col halves of the PE array
# stream two weight chunks concurrently.
COLPACK = False
USE_GPSIMD_T2 = False
USE_TTR = False


def _head(nc, ph, ptmp, h_st, hwb, hbcol, out_col):
    """pred = h @ head_w + head_b into out_col [B, 1], on DVE."""
    if USE_TTR:
        nc.vector.tensor_tensor_reduce(
            out=ph, in0=h_st, in1=hwb, scale=1.0,
            scalar=hbcol[:, 0:1], op0=mybir.AluOpType.mult,
            op1=mybir.AluOpType.add, accum_out=out_col)
    else:
        nc.vector.tensor_mul(ph, h_st, hwb)
        nc.vector.tensor_reduce(ptmp, ph, axis=mybir.AxisListType.X,
                                op=mybir.AluOpType.add)
        nc.vector.tensor_add(out_col, ptmp, hbcol)

PHASES = {}


def _mark(nc, name):
    PHASES[name] = int(nc.get_next_instruction_name().split("-")[-1])


def _mmr(nc, out, lhsT, rhs, start, stop):
    """fp32 matmul streamed in float32r mode (1 cycle/row at N>=256)."""
    nc.tensor.matmul(out, lhsT=lhsT.bitcast(F32R), rhs=rhs.bitcast(F32R),
                     start=start, stop=stop)


def build_nc(repeat=1, variant="full"):
    nc = bacc.Bacc(None, target_bir_lowering=False, debug=False)

    # ---------- DRAM I/O ----------
    x_d = nc.dram_tensor("x", [B, S, CIN], F32, kind="ExternalInput")
    dstart_d = nc.dram_tensor("decoder_start", [B, 1], F32, kind="ExternalInput")
    convw_d = nc.dram_tensor("conv_w", [OC, CIN, KW], F32, kind="ExternalInput")
    convb_d = nc.dram_tensor("conv_b", [OC], F32, kind="ExternalInput")
    encWih_d = nc.dram_tensor("enc_Wih", [G, OC], F32, kind="ExternalInput")
    encWhh_d = nc.dram_tensor("enc_Whh", [G, H], F32, kind="ExternalInput")
    encb_d = nc.dram_tensor("enc_b", [G], F32, kind="ExternalInput")
    decWih_d = nc.dram_tensor("dec_Wih", [G, 1], F32, kind="ExternalInput")
    decWhh_d = nc.dram_tensor("dec_Whh", [G, H], F32, kind="ExternalInput")
    decb_d = nc.dram_tensor("dec_b", [G], F32, kind="ExternalInput")
    headw_d = nc.dram_tensor("head_w", [1, H], F32, kind="ExternalInput")
    headb_d = nc.dram_tensor("head_b", [1], F32, kind="ExternalInput")
    out_d = nc.dram_tensor("out", [B, OUT_STEPS], F32, kind="ExternalOutput")

    with tile_mod.TileContext(nc) as tc:
        with tc.tile_pool(name="dram", bufs=1, space="DRAM") as dramp:
            # encoder inputs staged in DRAM as [t', b, oc]
            enc_x = dramp.tile([T2, B, OC], F32)

            with tc.tile_pool(name="const", bufs=1) as cn:
                identity = cn.tile([128, 128], F32)
                make_identity(nc, identity)
                id64 = identity[:64, :64]

                # persistent weights (stream operands)
                hW = [cn.tile([128, G], F32, name=f"hW{k}") for k in range(4)]
                xW = cn.tile([OC + 1, G], F32)          # rows 0..63 Wih.T, row 64 enc_b
                dhW = [cn.tile([128, G], F32, name=f"dhW{k}") for k in range(4)]
                dxW0 = cn.tile([1, G], F32)             # dWih.T-perm (decoder step-0 correction rhs)
                dbW = cn.tile([1, G], F32)              # db' = db + head_b*dWih (folded bias row)
                cwT = [cn.tile([CIN, OC], F32, name=f"cwT{k}") for k in range(KW)]
                cb = cn.tile([OC, 1], F32)
                ones_row = cn.tile([1, B], F32)
                ones_st = cn.tile([1, B], F32)
                nc.vector.memset(ones_st, 1.0)
                # f32r matmul operands must come from an f32r-rounding producer
                nc.vector.tensor_copy(ones_row.bitcast(F32R), ones_st)
                zpad = cn.tile([CIN, 2], F32)
                nc.vector.memset(zpad, 0.0)
                hwb = cn.tile([B, H], F32)              # head_w broadcast rows
                hbcol = cn.tile([B, 1], F32)            # head_b broadcast col
                dcol = cn.tile([B, 1], F32)             # decoder_start col
                pcol = cn.tile([B, 1], F32)             # pred_enc col
                dif = cn.tile([B, 1], F32)
                ph = cn.tile([B, H], F32)               # head product scratch
                ptmp = cn.tile([B, 1], F32)
                corr_row = cn.tile([1, B], F32)         # (dstart - pred_enc).T

                # persistent state
                c_st = cn.tile([B, H], F32)
                hT2 = [cn.tile([128, 4 * B], F32, name=f"hT{i}") for i in range(2)]
                sig = cn.tile([B, 3 * H], F32)          # sigmoid(f,i,o)
                sigFI2 = cn.tile([128, H], F32)         # colpack: [f; i] stacked
                tg = cn.tile([B, H], F32)
                tcell = cn.tile([B, H], F32)
                h_st = cn.tile([B, H], F32)
                t1 = cn.tile([B, H], F32)
                t2 = cn.tile([B, H], F32)
                outF = cn.tile([B, OUT_STEPS], F32)
                xbS = [cn.tile([B, OC], F32, name=f"xb{i}") for i in range(NXB)]
                xsTS = [cn.tile([OC + 1, B], F32, name=f"xsT{i}") for i in range(NXB)]
                for i in range(NXB):
                    nc.vector.tensor_copy(xsTS[i][OC:OC + 1, :].bitcast(F32R), ones_st)
                nc.vector.memset(c_st, 0.0)
                zT_st = cn.tile([128, 4 * B], F32)
                nc.vector.memset(zT_st, 0.0)
                nc.vector.tensor_copy(hT2[0].bitcast(F32R), zT_st)
                nc.vector.tensor_copy(hT2[1].bitcast(F32R), zT_st)
                nc.sync.dma_start(out=dcol, in_=dstart_d[:, :])

                # ---------- weight prep (on-chip transposes) ----------
                with (
                    tc.tile_pool(name="wtmp", bufs=3) as wt,
                    tc.tile_pool(name="wps", bufs=3, space="PSUM") as wps,
                ):
                    def prep_whh(src_d, dst_tiles):
                        for jb in range(16):
                            n = PERM.index(jb)
                            wtmp = wt.tile([128, H], F32, tag="wtmp")
                            nc.sync.dma_start(out=wtmp, in_=src_d[128 * jb:128 * (jb + 1), :])
                            for kc in range(4):
                                wtp = wps.tile([128, 128], F32, tag="wtp")
                                nc.tensor.transpose(wtp, wtmp[:, 128 * kc:128 * (kc + 1)], identity)
                                dst = dst_tiles[kc][:, 128 * n:128 * (n + 1)].bitcast(F32R)
                                if kc % 2 == 0:
                                    nc.scalar.copy(dst, wtp)
                                else:
                                    nc.vector.tensor_copy(dst, wtp)

                    prep_whh(encWhh_d, hW)
                    prep_whh(decWhh_d, dhW)

                    # enc_Wih.T into xW rows 0..63
                    for jb in range(16):
                        n = PERM.index(jb)
                        wtmp2 = wt.tile([128, OC], F32, tag="wtmp2")
                        nc.sync.dma_start(out=wtmp2, in_=encWih_d[128 * jb:128 * (jb + 1), :])
                        wtp = wps.tile([128, 128], F32, tag="wtp")
                        nc.tensor.transpose(wtp[:OC, :128], wtmp2, identity)
                        nc.scalar.copy(xW[0:OC, 128 * n:128 * (n + 1)].bitcast(F32R), wtp[:OC, :128])
                    # biases / vectors into fp32 staging (all at partition 0)
                    bstage = wt.tile([1, G], F32, tag="bstage", bufs=1)
                    dstage0 = wt.tile([1, G], F32, tag="dstage0", bufs=1)
                    dbstage = wt.tile([1, G], F32, tag="dbstage", bufs=1)
                    for n in range(16):
                        jb = PERM[n]
                        nc.sync.dma_start(out=bstage[:, 128 * n:128 * (n + 1)],
                                          in_=encb_d[None, 128 * jb:128 * (jb + 1)])
                        nc.sync.dma_start(out=dstage0[:, 128 * n:128 * (n + 1)],
                                          in_=decWih_d[128 * jb:128 * (jb + 1), :].rearrange("a b -> b a"))
                        nc.sync.dma_start(out=dbstage[:, 128 * n:128 * (n + 1)],
                                          in_=decb_d[None, 128 * jb:128 * (jb + 1)])
                    nc.scalar.copy(xW[OC:OC + 1, :].bitcast(F32R), bstage)

                    hwrow = wt.tile([1, H], F32, tag="hwrow", bufs=1)
                    nc.sync.dma_start(out=hwrow, in_=headw_d[:, :])
                    hbst = wt.tile([1, 1], F32, tag="hbst", bufs=1)
                    nc.sync.dma_start(out=hbst, in_=headb_d[:, None])

                    # fold head into decoder weights: db' = db + head_b * dWih
                    tmpb = wt.tile([1, G], F32, tag="tmpb", bufs=1)
                    dbrow = wt.tile([1, G], F32, tag="dbrow", bufs=1)
                    nc.vector.tensor_scalar_mul(tmpb, dstage0[:, :], hbst[0:1, 0:1])
                    nc.vector.tensor_add(dbrow, dbstage, tmpb)
                    nc.vector.tensor_copy(dxW0.bitcast(F32R), dstage0)
                    nc.vector.tensor_copy(dbW.bitcast(F32R), dbrow)

                    # dhW' = dWhh.T + head_w (x) dWih (outer-product fold)
                    for kc in range(4):
                        for n in range(4):
                            po = wps.tile([128, 512], F32, tag="po")
                            nc.tensor.matmul(
                                po,
                                lhsT=hwrow[:, 128 * kc:128 * (kc + 1)],
                                rhs=dstage0[:, 512 * n:512 * (n + 1)],
                                start=True, stop=True)
                            tfo = wt.tile([128, 512], F32, tag="tfo")
                            nc.vector.tensor_add(tfo, dhW[kc][:, 512 * n:512 * (n + 1)], po)
                            nc.scalar.copy(dhW[kc][:, 512 * n:512 * (n + 1)].bitcast(F32R), tfo)

                    # head_w / head_b broadcast across batch rows (K=1 matmuls)
                    pwb = wps.tile([B, H], F32, tag="po")
                    nc.tensor.matmul(pwb, lhsT=ones_st, rhs=hwrow, start=True, stop=True)
                    nc.vector.tensor_copy(hwb, pwb)
                    phb = wps.tile([B, 1], F32, tag="phb", bufs=1)
                    nc.tensor.matmul(phb, lhsT=ones_st, rhs=hbst, start=True, stop=True)
                    nc.vector.tensor_copy(hbcol, phb)

                    # conv weights: cwT[k][ic, oc] = conv_w[oc, ic, k]
                    cstage = wt.tile([CIN, KW * OC], F32, tag="cstage", bufs=1)
                    for k in range(KW):
                        nc.sync.dma_start(
                            out=cstage[:, OC * k:OC * (k + 1)],
                            in_=convw_d[:, :, k].rearrange("oc ic -> ic oc"),
                        )
                        nc.scalar.copy(cwT[k][:, :].bitcast(F32R), cstage[:, OC * k:OC * (k + 1)])
                    nc.sync.dma_start(out=cb, in_=convb_d[:, None])

                _mark(nc, "conv_start")
                # ---------- conv + pool -> enc_x ----------
                with nc.named_scope("conv"):
                    with (
                        tc.tile_pool(name="conv", bufs=2) as cp,
                        tc.tile_pool(name="convps", bufs=2, space="PSUM") as cpp,
                    ):
                        for b in range(B):
                            # xTb rows 0:16 hold x[b].T with 2-col zero pads; rows
                            # 16:32 are scratch written by the 32-partition unpack
                            # copies (PSUM reads must start 32-aligned).
                            xTb = cp.tile([32, S + 4 + 4], F32, tag="xTb")
                            nc.vector.tensor_copy(xTb[0:CIN, 0:2].bitcast(F32R), zpad)
                            nc.vector.tensor_copy(xTb[0:CIN, 2 + S:2 + S + 2].bitcast(F32R), zpad)
                            for half in range(2):
                                xb_raw = cp.tile([128, 128], F32, tag="xb_raw", bufs=3)
                                nc.sync.dma_start(
                                    out=xb_raw.rearrange("p (a c) -> p a c", c=32)[:, :, 0:CIN],
                                    in_=x_d[b].rearrange("(a p) c -> p a c", p=128)[:, 4 * half:4 * half + 4, :],
                                )
                                xtp = cpp.tile([128, 128], F32, tag="xtp")
                                nc.tensor.transpose(xtp, xb_raw, identity)
                                for a in range(4):
                                    blk = xtp[32 * a:32 * (a + 1), :]
                                    dst = xTb[:, 2 + 128 * (4 * half + a):2 + 128 * (4 * half + a + 1)].bitcast(F32R)
                                    if a % 2 == 0:
                                        nc.scalar.copy(dst, blk)
                                    else:
                                        nc.vector.tensor_copy(dst, blk)
                            # k-outer so both halves share each cwT[k] stationary
                            cps0 = cpp.tile([OC, 512], F32, tag="cps0")
                            cps1 = cpp.tile([OC, 512], F32, tag="cps1")
                            for k in range(KW):
                                _mmr(nc, cps0, cwT[k], xTb[0:CIN, k:k + 512], k == 0, k == KW - 1)
                                _mmr(nc, cps1, cwT[k], xTb[0:CIN, k + 512:k + 1024], k == 0, k == KW - 1)
                            yb = cp.tile([OC, S], F32, tag="yb")
                            nc.scalar.activation(yb[:, 0:512], cps0, AF.Relu, bias=cb[:, 0:1])
                            nc.scalar.activation(yb[:, 512:1024], cps1, AF.Relu, bias=cb[:, 0:1])
                            pooled = cp.tile([OC, T2], F32, tag="pooled")
                            yb_pairs = yb.rearrange("p (t two) -> p t two", two=2)
                            nc.vector.tensor_max(pooled, yb_pairs[:, :, 0], yb_pairs[:, :, 1])
                            poolT = cp.tile([128, 4 * OC], F32, tag="poolT")
                            for q in range(4):
                                ptp = cpp.tile([128, OC], F32, tag="ptp")
                                nc.tensor.transpose(ptp, pooled[:, 128 * q:128 * (q + 1)], id64)
                                if q % 2 == 0:
                                    nc.scalar.copy(poolT[:, OC * q:OC * (q + 1)], ptp)
                                else:
                                    nc.vector.tensor_copy(poolT[:, OC * q:OC * (q + 1)], ptp)
                            for q in range(4):
                                nc.sync.dma_start(
                                    out=enc_x[128 * q:128 * (q + 1), b, :],
                                    in_=poolT[:, OC * q:OC * (q + 1)],
                                )

                _mark(nc, "enc_start")
                # ---------- encoder + decoder ----------
                with (
                    tc.tile_pool(name="step", bufs=2) as sp,
                    tc.tile_pool(name="lps", bufs=1, space="PSUM") as lp,
                ):
                    gps = lp.tile([B, G], F32, tag="gates")

                    def emit_tr(qs, dst):
                        """Transpose h_st 128-col chunks qs into dst hT buffer."""
                        htp = lp.tile([128, 2 * B], F32, tag="htp", bufs=2)
                        for j, q in enumerate(qs):
                            nc.tensor.transpose(htp[:, B * j:B * (j + 1)],
                                                h_st[:, 128 * q:128 * (q + 1)], id64)
                        q0 = qs[0]
                        nc.vector.tensor_copy(
                            dst[:, B * q0:B * (q0 + 2)].bitcast(F32R), htp)

                    def groups(prv, first_lhsT, first_rhs, W, defer01=None, defer23=None,
                               extra=None):
                        """Gate matmuls, kc-major: [first][extra?][k0][k1][k2][k3].

                        first group carries the input/bias contribution and
                        start=True; k3 stops, bank order [f i g o] so the
                        f/i sigmoids can begin before the o bank stops."""
                        for n in range(4):
                            _mmr(nc, gps[:, 512 * n:512 * (n + 1)], first_lhsT,
                                 first_rhs[:, 512 * n:512 * (n + 1)], True, False)
                        if extra is not None:
                            xl, xr = extra
                            for n in range(4):
                                _mmr(nc, gps[:, 512 * n:512 * (n + 1)], xl,
                                     xr[:, 512 * n:512 * (n + 1)], False, False)
                        if defer01 is not None:
                            defer01()
                        for kc in range(2):
                            for n in range(4):
                                _mmr(nc, gps[:, 512 * n:512 * (n + 1)],
                                     prv[:, B * kc:B * (kc + 1)],
                                     W[kc][:, 512 * n:512 * (n + 1)], False, False)
                        if defer23 is not None:
                            defer23()
                        for n in range(4):
                            _mmr(nc, gps[:, 512 * n:512 * (n + 1)],
                                 prv[:, 2 * B:3 * B], W[2][:, 512 * n:512 * (n + 1)], False, False)
                        for n in (1, 2, 0, 3):
                            _mmr(nc, gps[:, 512 * n:512 * (n + 1)],
                                 prv[:, 3 * B:4 * B], W[3][:, 512 * n:512 * (n + 1)], False, True)

                    def elementwise():
                        """gates psum [g f i o] -> c_st, h_st (no transposes here;
                        both pairs are deferred into the next step's groups)."""
                        nc.scalar.activation(tg, gps[:, 0:H], AF.Tanh)
                        nc.scalar.activation(sig[:, 0:2 * H], gps[:, H:3 * H], AF.Sigmoid)
                        nc.vector.tensor_mul(t1, sig[:, 0:H], c_st)
                        if USE_GPSIMD_T2:
                            nc.gpsimd.tensor_mul(t2, sig[:, H:2 * H], tg)
                        else:
                            nc.vector.tensor_mul(t2, sig[:, H:2 * H], tg)
                        nc.vector.tensor_add(c_st, t1, t2)
                        for hh in range(2):
                            sl = slice(HH * hh, HH * (hh + 1))
                            so = slice(2 * H + HH * hh, 2 * H + HH * (hh + 1))
                            go = slice(3 * H + HH * hh, 3 * H + HH * (hh + 1))
                            nc.scalar.activation(sig[:, so], gps[:, go], AF.Sigmoid)
                            nc.scalar.activation(tcell[:, sl], c_st[:, sl], AF.Tanh)
                            nc.vector.tensor_mul(h_st[:, sl], sig[:, so], tcell[:, sl])

                    # --- encoder loop ---
                    with nc.named_scope("enc"):
                        for t in range(T2):
                            slot = t % NXB
                            nc.sync.dma_start(out=xbS[slot], in_=enc_x[t])
                            xps = lp.tile([OC, B], F32, tag="small", bufs=2)
                            nc.tensor.transpose(xps, xbS[slot], id64)
                            nc.vector.tensor_copy(xsTS[slot][0:OC, :].bitcast(F32R), xps)
                            prv = hT2[(t + 1) % 2]
                            d01 = (lambda pv=prv: emit_tr((0, 1), pv)) if t > 0 else None
                            d23 = (lambda pv=prv: emit_tr((2, 3), pv)) if t > 0 else None
                            groups(prv, xsTS[slot], xW, hW, d01, d23)
                            elementwise()

                    _mark(nc, "dec_start")
                    with nc.named_scope("dec"):
                        # pred_enc and the step-0 correction row
                        _head(nc, ph, ptmp, h_st, hwb, hbcol, pcol)
                        nc.vector.tensor_sub(dif, dcol, pcol)
                        dps = lp.tile([1, B], F32, tag="small", bufs=2)
                        nc.tensor.transpose(dps, dif, id64)
                        nc.vector.tensor_copy(corr_row.bitcast(F32R), dps)

                        # --- decoder loop (folded weights) ---
                        for d in range(OUT_STEPS):
                            s = T2 + d
                            prv = hT2[(s + 1) % 2]
                            extra = (corr_row, dxW0) if d == 0 else None
                            d01 = lambda pv=prv: emit_tr((0, 1), pv)
                            d23 = lambda pv=prv: emit_tr((2, 3), pv)
                            groups(prv, ones_row, dbW, dhW, d01, d23, extra=extra)
                            elementwise()
                            _head(nc, ph, ptmp, h_st, hwb, hbcol, outF[:, d:d + 1])

                        nc.sync.dma_start(out=out_d[:, :], in_=outF)

    _mark(nc, "end")
    nc.compile()
    return nc


_CACHED = {}


def kernel(**inputs):
    """Full-input entry: shard batch across 8 cores, run SPMD, gather."""
    from concourse.bass_utils import run_bass_kernel_spmd

    if "nc" not in _CACHED:
        _CACHED["nc"] = build_nc()
    nc = _CACHED["nc"]

    full = {k: np.ascontiguousarray(np.asarray(v, dtype=np.float32)) for k, v in inputs.items()}
    per_core = []
    for c in range(NCORES):
        sl = slice(c * B, (c + 1) * B)
        m = {}
        for k, v in full.items():
            if k in ("x", "decoder_start"):
                m[k] = np.ascontiguousarray(v[sl])
            else:
                m[k] = v
        per_core.append(m)

    res = run_bass_kernel_spmd(nc, per_core, core_ids=list(range(NCORES)))
    outs = [r["out"] for r in res.results]
    return np.concatenate(outs, axis=0)
